# revision 35
# baseline (speedup 1.0000x reference)
"""BiGCN (two fused GCNConv + graph mean-pool + FC + log_softmax) on 8 trn2 cores.

Strategy (graph/data parallel, partitioned by destination node range):
  - core c owns nodes [c*NSH, (c+1)*NSH) as edge destinations
  - host sorts edges into per-(bank, dst-tile) cells padded to 128-slot
    chunks; SWDGE dma_gather per cell with per-core real counts fed via
    gpsimd registers (the ~9ns/idx gpsimd ucode is the hw floor for
    row-granular gathers; HW-DGE queues cannot expand indirect offsets)
  - host precomputes degree-normalization (dinv), pooling one-hots and
    per-graph inverse counts; x is shipped pre-transposed in bf16
  - device: Hn = (xT.T @ [W_td|W_bu]) * dinv (bf16) written per bank
    stripe; bank 0 is small (5 tiles) so its AllGather lands early and
    gathers start ~40us in; per-bank waves gather + scatter-accumulate
    into an SBUF f32 accumulator via one-hot matmuls (one-hot built on
    DVE with packed-last-dim APs); each tile's normalize/feat/pool work
    is fused right after its last bank so the tail hides in the gathers
  - final: out[d] = acc[d]*dinv[d] + (h[d]*dinv[d]^2 + b);
    feat = [relu(td)|relu(bu) | td|bu] (fc_W rows host-permuted),
    graph pooling via host one-hot matmuls, indirect-scatter +
    AllReduce, FC + log_softmax replicated on every core.
"""

import math

import numpy as np
import ml_dtypes

import concourse.bass as bass
import concourse.bacc as bacc
import concourse.mybir as mybir
import concourse.tile as tile
from concourse.bass import IndirectOffsetOnAxis
from concourse.bass_utils import run_bass_kernel_spmd
from concourse.library_config import mlp as mlp_lib

BF16 = mybir.dt.bfloat16
F32 = mybir.dt.float32
I16 = mybir.dt.int16
I32 = mybir.dt.int32
AF = mybir.ActivationFunctionType
ALU = mybir.AluOpType
NPBF = ml_dtypes.bfloat16

P = 128


def _split_even(n, k):
    base = n // k
    rem = n % k
    return [base + (1 if i < rem else 0) for i in range(k)]


class Cfg:
    def __init__(self, n_nodes, n_graphs, n_cores, banks, in_f, hid_f, out_f,
                 grp=14):
        assert n_nodes % n_cores == 0
        self.N = n_nodes
        self.G = n_graphs
        self.NC = n_cores
        self.NSH = n_nodes // n_cores
        self.T = math.ceil(self.NSH / P)
        self.NSH_P = self.T * P
        self.BANKS = min(banks, self.T)
        # small leading stripe so bank-0 gathers start early; later stripes
        # at the int16 table-index cap (n_cores*cap*128 <= 32767)
        cap = 32767 // (n_cores * P)
        if (self.BANKS > 2
                and (self.BANKS - 1) * cap < self.T <= self.BANKS * cap
                and self.T - (self.BANKS - 1) * cap + 2 <= cap):
            lead = self.T - (self.BANKS - 1) * cap + 2
            self.QT = ([lead] + [cap] * (self.BANKS - 2) + [cap - 2])
        elif (self.BANKS > 1
                and (self.BANKS - 1) * cap < self.T <= self.BANKS * cap):
            lead = self.T - (self.BANKS - 1) * cap
            self.QT = [lead] + [cap] * (self.BANKS - 1)
        else:
            self.QT = _split_even(self.T, self.BANKS)  # tiles per stripe
        self.QSTART = np.concatenate([[0], np.cumsum(self.QT)])
        self.QROWS = [q * P for q in self.QT]
        self.IN_F = in_f
        self.HID = hid_f
        self.FW = 2 * hid_f
        assert self.FW == P and in_f == P
        self.OUT_F = out_f
        self.FEAT = 4 * hid_f  # [relu(td)|relu(bu) | td|bu]
        self.GB = math.ceil(self.G / P)
        self.PART_ROWS = (self.G + 2 * P + P - 1) // P * P
        self.GRP = min(grp, self.T)
        self.NGRP = math.ceil(self.T / self.GRP)


def host_prep(cfg, x, edge_index, batch):
    """Sort edges into (bank, dst-tile) cells, build device inputs."""
    c = cfg
    src = edge_index[0].astype(np.int64)
    dst = edge_index[1].astype(np.int64)
    assert src.min() >= 0 and src.max() < c.N
    assert dst.min() >= 0 and dst.max() < c.N

    # degree incl. self-loop, symmetric normalization
    deg = np.bincount(dst, minlength=c.N).astype(np.float64) + 1.0
    dinv = 1.0 / np.sqrt(deg)  # [N]

    # source row in its bank table (bank k rows: core-major stripes)
    qstart_rows = c.QSTART[:-1] * P
    sc, so = np.divmod(src, c.NSH)
    stile = so // P
    sbank = np.searchsorted(c.QSTART[1:], stile, side="right")
    qrows_arr = np.asarray(c.QROWS)
    lidx = (sc * qrows_arr[sbank] + (so - qstart_rows[sbank])).astype(np.int64)

    owner, do = np.divmod(dst, c.NSH)
    tloc = do // P
    dl = do % P

    # cells ordered (owner, bank, tile); packed at 16-slot granules
    ncell = c.NC * c.BANKS * c.T
    cell = (owner * c.BANKS + sbank) * c.T + tloc
    order = np.argsort(cell, kind="stable")
    cell_s = cell[order]
    lidx_s = lidx[order]
    dl_s = dl[order]
    tloc_s = tloc[order]
    counts = np.bincount(cell_s, minlength=ncell).reshape(c.NC, c.BANKS, c.T)
    # cell capacity in slots: 16-granular, >=128 so a 128-chunk never spans
    # more than two (adjacent-tile) cells
    S = np.maximum((-(-counts // 16)).max(axis=0) * 16, P)  # [BANKS, T]

    starts = np.zeros(ncell + 1, dtype=np.int64)
    np.cumsum(counts.reshape(-1), out=starts[1:])
    rank = np.arange(len(cell_s), dtype=np.int64) - starts[cell_s]

    # calls: (bank, tile-group); cells packed back-to-back inside a call
    calls = []  # dicts: j, t0, t1, L, CH, slot_base, ebase, dbase, win, o
    slot_base = 0
    ecols = 0
    dcols = 0
    for j in range(c.BANKS):
        t = 0
        while t < c.T:
            t1 = min(t + c.GRP, c.T)
            o = np.zeros(t1 - t + 1, dtype=np.int64)
            np.cumsum(S[j, t:t1], out=o[1:])
            L = int(o[-1])
            CH = -(-L // P)
            # window base tile per chunk: the cell containing slot q*128
            win = np.searchsorted(o[1:], np.arange(CH) * P, side="right") + t
            calls.append(dict(j=j, t0=t, t1=t1, L=L, CH=CH,
                              slot_base=slot_base, ebase=ecols, dbase=dcols,
                              win=win, o=o))
            slot_base += CH * P  # call slot spaces are chunk-padded
            ecols += L // 16
            dcols += CH
            t = t1
    SLOT_TOT = slot_base
    ECOLS = max(ecols, 8)
    DCOLS = max(dcols, 1)
    CHMAX = max((cl["CH"] for cl in calls), default=1)

    # per-edge global slot position and window-relative dst id
    cell_jt = cell_s % (c.BANKS * c.T)
    jj = cell_jt // c.T
    tt_ = cell_jt % c.T
    call_of_tile = np.zeros((c.BANKS, c.T), dtype=np.int64)
    for ci, cl in enumerate(calls):
        call_of_tile[cl["j"], cl["t0"]:cl["t1"]] = ci
    call_id = call_of_tile[jj, tt_]
    cbase = np.array([cl["slot_base"] for cl in calls], dtype=np.int64)
    ct0 = np.array([cl["t0"] for cl in calls], dtype=np.int64)
    ocum = np.zeros((c.BANKS, c.T), dtype=np.int64)  # cell offset in call
    for cl in calls:
        ocum[cl["j"], cl["t0"]:cl["t1"]] = cl["o"][:-1]
    slot_in_call = ocum[jj, tt_] + rank
    slotpos = cbase[call_id] + slot_in_call
    winflat = np.concatenate([cl["win"] for cl in calls]) if calls else \
        np.zeros(1, np.int64)
    chbase = np.zeros(len(calls), dtype=np.int64)
    acc_ch = 0
    for ci, cl in enumerate(calls):
        chbase[ci] = acc_ch
        acc_ch += cl["CH"]
    ta_of_edge = winflat[chbase[call_id] + slot_in_call // P]
    dl2_s = dl_s + P * (tloc_s - ta_of_edge)
    assert dl2_s.min() >= 0 and dl2_s.max() < 2 * P, "chunk window overflow"

    g_base = np.empty(c.NC, dtype=np.int64)
    cnt_g = np.bincount(batch, minlength=c.G).astype(np.float64)

    per_core = []
    for cc in range(c.NC):
        e0, e1 = starts[cc * c.BANKS * c.T], starts[(cc + 1) * c.BANKS * c.T]
        eflat = np.full(SLOT_TOT, -1, dtype=np.int64)
        dflat = np.full(SLOT_TOT, 500.0, dtype=np.float32)
        eflat[slotpos[e0:e1]] = lidx_s[e0:e1]
        dflat[slotpos[e0:e1]] = dl2_s[e0:e1]
        # dl2 per (partition, chunk-col): [P, DCOLS]
        dlh = np.ascontiguousarray(
            dflat.reshape(-1, P).T
        ).astype(NPBF)

        eidx16 = np.full((P, ECOLS), -1, dtype=np.int16)
        nreal = np.zeros((1, -(-max(len(calls), 1) // 4) * 4),
                         dtype=np.int32)
        for ci, cl in enumerate(calls):
            sb, L = cl["slot_base"], cl["L"]
            li = eflat[sb : sb + L].copy()
            nz = np.flatnonzero(li != -1)
            if len(nz) == 0:
                li[0] = 0
                n = 1
            else:
                # mid pads must be valid indices (ucode forbids interior
                # -1); point them at row 0, zero-weighted by the one-hot
                last = int(nz[-1])
                seg = li[: last + 1]
                seg[seg == -1] = 0
                n = last + 1
            w = li.reshape(L // 16, 16).T.astype(np.int16)
            eb = cl["ebase"]
            eidx16[:, eb : eb + L // 16] = np.tile(w, (8, 1))
            nreal[0, ci] = n

        xs = np.zeros((c.NSH_P, c.IN_F), dtype=np.float32)
        xs[: c.NSH] = x[cc * c.NSH : (cc + 1) * c.NSH]
        xT = np.ascontiguousarray(xs.T).astype(NPBF)  # [IN_F, NSH_P]

        dv = np.zeros(c.NSH_P, dtype=np.float32)
        dv[: c.NSH] = dinv[cc * c.NSH : (cc + 1) * c.NSH]
        dinvT = np.ascontiguousarray(dv.reshape(c.T, P).T)
        dinv2T = dinvT * dinvT

        b = batch[cc * c.NSH : (cc + 1) * c.NSH]
        g_base[cc] = int(b[0])
        assert int(b[-1]) - int(b[0]) < 2 * P, "graph span exceeds 2 blocks"
        brel = np.full(c.NSH_P, 300, dtype=np.int64)
        brel[: c.NSH] = b - g_base[cc]
        pohg = np.zeros((c.NSH_P, 2 * P), dtype=np.float32)
        valid = brel < 2 * P
        pohg[np.arange(c.NSH_P)[valid], brel[valid]] = 1.0
        # [P, T*2P]: tile-major blocks, slot p on partition p
        pohgT = np.ascontiguousarray(
            pohg.reshape(c.T, P, 2 * P).transpose(1, 0, 2).reshape(P, -1)
        ).astype(NPBF)

        goff0 = (g_base[cc] + np.arange(P)).astype(np.int32).reshape(P, 1)
        goff1 = goff0 + P
        per_core.append(
            dict(xT=xT, eidx16=eidx16, nreal=nreal, dlh=dlh, pohg=pohgT,
                 dinvT=dinvT, dinv2T=dinv2T, goff0=goff0, goff1=goff1)
        )

    # shared constants
    iotaC = np.repeat(np.arange(2 * P, dtype=np.float32), CHMAX)
    iotaC = np.tile(iotaC, (P, 1)).astype(NPBF)  # [P, 2P*CHMAX]
    invc = np.ones((P, c.GB), dtype=np.float32)
    for bb in range(c.GB):
        gs = np.arange(bb * P, min((bb + 1) * P, c.G))
        invc[: len(gs), bb] = 1.0 / np.maximum(cnt_g[gs], 1.0)
    ident32 = np.eye(P, dtype=np.float32)

    meta = dict(calls=calls, S=S, ECOLS=ECOLS, DCOLS=DCOLS, CHMAX=CHMAX,
                iotaC=iotaC, invc=invc, ident32=ident32)
    return meta, per_core


def build_program(cfg, meta, debug=False):
    c = cfg
    calls = meta["calls"]
    ECOLS = meta["ECOLS"]
    DCOLS = meta["DCOLS"]
    CHMAX = meta["CHMAX"]
    H = c.HID
    FW = c.FW
    FEAT = c.FEAT
    NCALL = -(-max(len(calls), 1) // 4) * 4

    nc = bacc.Bacc(
        "TRN2", target_bir_lowering=False, debug=debug, num_devices=c.NC
    )

    # ---- I/O ----
    xT_d = nc.dram_tensor("xT", [P, c.NSH_P], BF16, kind="ExternalInput")
    wcat_d = nc.dram_tensor("wcat", [c.IN_F, FW], BF16, kind="ExternalInput")
    bias_d = nc.dram_tensor("bias_bc", [P, FW], BF16, kind="ExternalInput")
    dinvT_d = nc.dram_tensor("dinvT", [P, c.T], F32, kind="ExternalInput")
    dinv2T_d = nc.dram_tensor("dinv2T", [P, c.T], F32, kind="ExternalInput")
    eidx_d = nc.dram_tensor("eidx16", [P, ECOLS], I16, kind="ExternalInput")
    nreal_d = nc.dram_tensor("nreal", [1, NCALL], I32, kind="ExternalInput")
    dl_d = nc.dram_tensor("dlh", [P, DCOLS], BF16, kind="ExternalInput")
    iota_d = nc.dram_tensor("iotaC", [P, 2 * P * CHMAX], BF16,
                            kind="ExternalInput")
    pohg_d = nc.dram_tensor("pohg", [P, c.T * 2 * P], BF16, kind="ExternalInput")
    goff0_d = nc.dram_tensor("goff0", [P, 1], I32, kind="ExternalInput")
    goff1_d = nc.dram_tensor("goff1", [P, 1], I32, kind="ExternalInput")
    invc_d = nc.dram_tensor("invc", [P, c.GB], F32, kind="ExternalInput")
    fcW_d = nc.dram_tensor("fc_W", [2 * FW, c.OUT_F], F32, kind="ExternalInput")
    fcb_d = nc.dram_tensor("fc_b", [c.OUT_F], F32, kind="ExternalInput")
    ident_d = nc.dram_tensor("ident32", [P, P], F32, kind="ExternalInput")
    out = nc.dram_tensor("out", [c.G, c.OUT_F], F32, kind="ExternalOutput")

    # ---- internal DRAM ----
    hn_local = nc.dram_tensor("hn_local", [c.NSH_P, FW], BF16)
    hn_q = [
        nc.dram_tensor(f"hn_q{k}", [c.NC * c.QROWS[k], FW], BF16,
                       addr_space="Shared")
        for k in range(c.BANKS)
    ]
    partial = nc.dram_tensor("partial", [c.PART_ROWS, FEAT], BF16)
    total = nc.dram_tensor("total", [c.PART_ROWS, FEAT], BF16,
                           addr_space="Shared")

    groups = [list(range(c.NC))]

    with tile.TileContext(nc) as tc:
        with (
            tc.tile_pool(name="const", bufs=1) as cp,
            tc.tile_pool(name="sb", bufs=3) as sp,
            tc.tile_pool(name="oh", bufs=2) as op_,
            tc.tile_pool(name="strip", bufs=4) as gp,
            nc.gpsimd.register("nr0") as r0,
            nc.gpsimd.register("nr1") as r1,
            nc.gpsimd.register("nr2") as r2,
            nc.gpsimd.register("nr3") as r3,
        ):
            regs = [r0, r1, r2, r3]
            nc.gpsimd.load_library(mlp_lib)

            # ---- constants ----
            wcat = cp.tile([P, FW], BF16)
            nc.sync.dma_start(wcat[:], wcat_d[:])
            bias_sb = cp.tile([P, FW], BF16)
            nc.sync.dma_start(bias_sb[:], bias_d[:])
            dinvT = cp.tile([P, c.T], F32)
            nc.sync.dma_start(dinvT[:], dinvT_d[:])
            dinv2T = cp.tile([P, c.T], F32)
            nc.sync.dma_start(dinv2T[:], dinv2T_d[:])
            eidx_sb = cp.tile([P, ECOLS], I16)
            nc.scalar.dma_start(eidx_sb[:], eidx_d[:])
            nreal_sb = cp.tile([1, NCALL], I32)
            nc.scalar.dma_start(nreal_sb[:], nreal_d[:])
            dl_sb = cp.tile([P, DCOLS], BF16)
            nc.scalar.dma_start(dl_sb[:], dl_d[:])
            iota_sb = cp.tile([P, 2 * P * CHMAX], BF16)
            nc.scalar.dma_start(iota_sb[:], iota_d[:])
            goff0_sb = cp.tile([P, 1], I32)
            nc.sync.dma_start(goff0_sb[:], goff0_d[:])
            goff1_sb = cp.tile([P, 1], I32)
            nc.sync.dma_start(goff1_sb[:], goff1_d[:])
            invc_sb = cp.tile([P, c.GB], F32)
            nc.sync.dma_start(invc_sb[:], invc_d[:])
            fw0 = cp.tile([P, c.OUT_F], F32)
            nc.sync.dma_start(fw0[:], fcW_d[0:P, :])
            fw1 = cp.tile([P, c.OUT_F], F32)
            nc.sync.dma_start(fw1[:], fcW_d[P : 2 * P, :])
            fcb = cp.tile([c.OUT_F, 1], F32)
            nc.sync.dma_start(fcb[:, 0:1], fcb_d[:, None])
            ident32 = cp.tile([P, P], F32)
            nc.sync.dma_start(ident32[:], ident_d[:])

            hb_all = cp.tile([P, c.T * FW], BF16)   # h*dinv^2 + bias
            acc_all = cp.tile([P, c.T * FW], BF16)  # edge-message sums

            # ---- phase 1: Hn per bank stripe, AllGather each stripe ----
            with tc.tile_pool(name="ps1", bufs=2, space="PSUM") as pp:
                for k in range(c.BANKS):
                    for t in range(int(c.QSTART[k]), int(c.QSTART[k + 1])):
                        xt = sp.tile([P, P], BF16, tag="xt")
                        nc.sync.dma_start(xt[:], xT_d[:, t * P : (t + 1) * P])
                        h_ps = pp.tile([P, FW], F32, space="PSUM", tag="h")
                        nc.tensor.matmul(
                            h_ps[:], lhsT=xt[:], rhs=wcat[:],
                            start=True, stop=True,
                        )
                        hn_sb = sp.tile([P, FW], BF16, tag="hn")
                        nc.scalar.activation(
                            hn_sb[:], h_ps[:], AF.Copy,
                            scale=dinvT[:, t : t + 1],
                        )
                        nc.sync.dma_start(
                            hn_local[t * P : (t + 1) * P, :], hn_sb[:]
                        )
                        hb0 = sp.tile([P, FW], BF16, tag="hb0")
                        nc.scalar.activation(
                            hb0[:], h_ps[:], AF.Copy,
                            scale=dinv2T[:, t : t + 1],
                        )
                        nc.vector.tensor_tensor(
                            out=hb_all[:, t * FW : (t + 1) * FW],
                            in0=hb0[:], in1=bias_sb[:], op=ALU.add,
                        )
                    r_lo = int(c.QSTART[k]) * P
                    nc.gpsimd.collective_compute(
                        "AllGather",
                        ALU.bypass,
                        ins=[hn_local[r_lo : r_lo + c.QROWS[k], :]],
                        outs=[hn_q[k][:]],
                        replica_groups=groups,
                    )

            # zero the pooling partial buffer early (SP queue, off critical path)
            zt = sp.tile([P, FEAT], BF16, tag="zt")
            nc.vector.memset(zt[:], 0.0)
            for r in range(0, c.PART_ROWS, P):
                nc.sync.dma_start(partial[r : r + P, :], zt[:])

            # ---- scatter waves: per bank, merged tile-group gather calls;
            # ---- per-tile normalize/feat/pool fused after its last bank ----
            first = [True] * c.T
            fin_first = 0
            fin_last = c.T - 1

            def finalize_tile(t):
                poh = sp.tile([P, 2 * P], BF16, tag="poh")
                nc.sync.dma_start(
                    poh[:], pohg_d[:, t * 2 * P : (t + 1) * 2 * P]
                )
                s1t = sp.tile([P, FW], F32, tag="s1")
                nc.scalar.activation(
                    s1t[:], acc_all[:, t * FW : (t + 1) * FW],
                    AF.Copy, scale=dinvT[:, t : t + 1],
                )
                ot = sp.tile([P, FW], F32, tag="ot")
                nc.vector.tensor_tensor(
                    out=ot[:], in0=s1t[:],
                    in1=hb_all[:, t * FW : (t + 1) * FW], op=ALU.add,
                )
                feat = sp.tile([P, FEAT], BF16, tag="feat")
                nc.scalar.activation(feat[:, 0:FW], ot[:], AF.Relu)
                nc.scalar.activation(feat[:, FW:FEAT], ot[:], AF.Copy)
                nc.tensor.matmul(
                    pool0[:], lhsT=poh[:, 0:P], rhs=feat[:],
                    start=(t == fin_first), stop=(t == fin_last),
                )
                nc.tensor.matmul(
                    pool1[:], lhsT=poh[:, P : 2 * P], rhs=feat[:],
                    start=(t == fin_first), stop=(t == fin_last),
                )

            n_gather = 0
            GT_BUFS = 4
            with (
                tc.tile_pool(name="psP", bufs=1, space="PSUM") as pa,
                tc.tile_pool(name="ps4", bufs=3, space="PSUM") as pp,
            ):
                pool0 = pa.tile([P, FEAT], F32, space="PSUM")
                pool1 = pa.tile([P, FEAT], F32, space="PSUM")
                for ci, cl in enumerate(calls):
                    j, t0, t1 = cl["j"], cl["t0"], cl["t1"]
                    L, CH = cl["L"], cl["CH"]
                    o, win = cl["o"], cl["win"]
                    gt_t = gp.tile([P, CHMAX * P], BF16, tag="gt")
                    # pad slots are never written by the gather: scrub each
                    # pool slot once at uniform max extent; afterwards every
                    # byte is scrub-zero or old gather data (finite), so no
                    # NaN reaches the zero-weighted matmuls
                    if n_gather < GT_BUFS:
                        nc.vector.memset(gt_t[:], 0.0)
                    if n_gather % 4 == 0:
                        nc.gpsimd.reg_load(
                            regs, nreal_sb[0:1, ci : ci + 4]
                        )
                    reg = regs[n_gather % 4]
                    eb = cl["ebase"]
                    nc.gpsimd.dma_gather(
                        gt_t[:, 0 : CH * P].rearrange(
                            "p (b e) -> p b e", e=FW
                        ),
                        hn_q[j][:],
                        eidx_sb[:, eb : eb + L // 16],
                        L,
                        reg,
                        FW,
                        single_packet=(L <= 1024),
                    )
                    n_gather += 1
                    db = cl["dbase"]
                    oh = op_.tile([P, 2 * P * CH], BF16, tag="oh")
                    nc.vector.tensor_tensor(
                        out=oh[:].rearrange("p (d g) -> p d g", g=CH),
                        in0=dl_sb[:, db : db + CH]
                        .unsqueeze(1)
                        .broadcast_to([P, 2 * P, CH]),
                        in1=iota_sb[:]
                        .rearrange("p (d g) -> p d g", g=CHMAX)[:, :, 0:CH],
                        op=ALU.is_equal,
                    )
                    oh3 = oh[:].rearrange("p (d g) -> p d g", g=CH)
                    for t in range(t0, t1):
                        q_lo = int(o[t - t0]) // P
                        q_hi = (int(o[t - t0 + 1]) - 1) // P
                        acc = pp.tile([P, FW], F32, space="PSUM", tag="acc")
                        for q in range(q_lo, q_hi + 1):
                            assert win[q] in (t, t - 1), (win[q], t)
                            half = 0 if win[q] == t else 1
                            nc.tensor.matmul(
                                acc[:],
                                lhsT=oh3[:, half * P : (half + 1) * P,
                                         q : q + 1],
                                rhs=gt_t[:, q * P : (q + 1) * P],
                                start=(q == q_lo),
                                stop=(q == q_hi),
                            )
                        dst_sl = acc_all[:, t * FW : (t + 1) * FW]
                        if first[t]:
                            nc.vector.tensor_copy(dst_sl, acc[:])
                            first[t] = False
                        else:
                            nc.vector.tensor_tensor(
                                out=dst_sl, in0=acc[:], in1=dst_sl,
                                op=ALU.add,
                            )
                        if j == c.BANKS - 1:
                            finalize_tile(t)

                pp0 = sp.tile([P, FEAT], BF16, tag="pp0")
                nc.vector.tensor_copy(pp0[:], pool0[:])
                nc.gpsimd.indirect_dma_start(
                    out=partial[:],
                    out_offset=IndirectOffsetOnAxis(ap=goff0_sb[:, 0:1],
                                                    axis=0),
                    in_=pp0[:],
                    in_offset=None,
                )
                pp1 = sp.tile([P, FEAT], BF16, tag="pp1")
                nc.vector.tensor_copy(pp1[:], pool1[:])
                nc.gpsimd.indirect_dma_start(
                    out=partial[:],
                    out_offset=IndirectOffsetOnAxis(ap=goff1_sb[:, 0:1],
                                                    axis=0),
                    in_=pp1[:],
                    in_offset=None,
                )

            # ---- AllReduce pooled sums ----
            nc.gpsimd.collective_compute(
                "AllReduce",
                ALU.add,
                ins=[partial[:]],
                outs=[total[:]],
                replica_groups=groups,
            )

            # ---- mean, FC, log_softmax (replicated) ----
            # two passes so the scalar engine loads the Exp/Ln activation
            # tables once instead of per graph-block
            OUTF = c.OUT_F
            ls_all = cp.tile([P, c.GB * OUTF], F32)
            ex_all = cp.tile([P, c.GB * OUTF], F32)
            sm_all = cp.tile([P, c.GB], F32)
            with tc.tile_pool(name="ps7", bufs=2, space="PSUM") as pp:
                for b in range(c.GB):
                    tt = sp.tile([P, FEAT], BF16, tag="tt")
                    nc.sync.dma_start(tt[:], total[b * P : (b + 1) * P, :])
                    mean_sb = sp.tile([P, FEAT], F32, tag="mean")
                    nc.vector.tensor_scalar(
                        out=mean_sb[:], in0=tt[:],
                        scalar1=invc_sb[:, b : b + 1], scalar2=None,
                        op0=ALU.mult,
                    )
                    lg_ps = pp.tile([P, P], F32, space="PSUM", tag="lg")
                    for half in range(2):
                        tp_ps = pp.tile([P, P], F32, space="PSUM", tag="tp")
                        nc.tensor.transpose(
                            tp_ps[:], mean_sb[:, half * P : (half + 1) * P],
                            ident32[:],
                        )
                        mt = sp.tile([P, P], F32, tag="mt")
                        nc.vector.tensor_copy(mt[:], tp_ps[:])
                        nc.tensor.matmul(
                            lg_ps[0:OUTF, :],
                            lhsT=(fw0 if half == 0 else fw1)[:],
                            rhs=mt[:],
                            start=(half == 0),
                            stop=(half == 1),
                        )
                    lgb = sp.tile([OUTF, P], F32, tag="lgb")
                    nc.vector.tensor_scalar(
                        out=lgb[:], in0=lg_ps[0:OUTF, :],
                        scalar1=fcb[:, 0:1], scalar2=None, op0=ALU.add,
                    )
                    tr_ps = pp.tile([P, OUTF], F32, space="PSUM", tag="tr")
                    nc.tensor.transpose(
                        tr_ps[:], lgb[:], ident32[0:OUTF, 0:OUTF]
                    )
                    sl = ls_all[:, b * OUTF : (b + 1) * OUTF]
                    nc.vector.tensor_copy(sl, tr_ps[:])
                    mx = sp.tile([P, 1], F32, tag="mx")
                    nc.vector.reduce_max(mx[:], sl, axis=mybir.AxisListType.X)
                    nc.vector.tensor_scalar(
                        out=sl, in0=sl, scalar1=mx[:, 0:1], scalar2=None,
                        op0=ALU.subtract,
                    )
                for b in range(c.GB):
                    nc.scalar.activation(
                        ex_all[:, b * OUTF : (b + 1) * OUTF],
                        ls_all[:, b * OUTF : (b + 1) * OUTF], AF.Exp,
                    )
                for b in range(c.GB):
                    nc.vector.reduce_sum(
                        sm_all[:, b : b + 1],
                        ex_all[:, b * OUTF : (b + 1) * OUTF],
                        axis=mybir.AxisListType.X,
                    )
                for b in range(c.GB):
                    nc.scalar.activation(
                        sm_all[:, b : b + 1], sm_all[:, b : b + 1], AF.Ln
                    )
                for b in range(c.GB):
                    h_rows = min(P, c.G - b * P)
                    ls = sp.tile([P, OUTF], F32, tag="ls")
                    nc.vector.tensor_scalar(
                        out=ls[:], in0=ls_all[:, b * OUTF : (b + 1) * OUTF],
                        scalar1=sm_all[:, b : b + 1], scalar2=None,
                        op0=ALU.subtract,
                    )
                    nc.sync.dma_start(
                        out[b * P : b * P + h_rows, :], ls[0:h_rows, :]
                    )

    nc.compile()
    return nc


def make_in_maps(cfg, meta, per_core, W_td, b_td, W_bu, b_bu, fc_W, fc_b):
    c = cfg
    H = c.HID
    wcat = np.concatenate(
        [np.asarray(W_td, np.float32), np.asarray(W_bu, np.float32)], axis=1
    ).astype(NPBF)
    bias = np.concatenate(
        [np.asarray(b_td, np.float32), np.asarray(b_bu, np.float32)]
    )
    bias_bc = np.tile(bias.reshape(1, -1), (P, 1)).astype(NPBF)
    # feat order [relu_td, relu_bu, td, bu] -> permute fc_W rows
    fw = np.asarray(fc_W, np.float32)
    perm = np.concatenate([
        np.arange(0, H),            # relu(td)
        np.arange(2 * H, 3 * H),    # relu(bu)
        np.arange(H, 2 * H),        # td
        np.arange(3 * H, 4 * H),    # bu
    ])
    fc_Wp = np.ascontiguousarray(fw[perm])

    in_maps = []
    for cc in range(c.NC):
        pc = per_core[cc]
        in_maps.append(
            {
                "xT": pc["xT"],
                "wcat": wcat,
                "bias_bc": bias_bc,
                "dinvT": pc["dinvT"],
                "dinv2T": pc["dinv2T"],
                "eidx16": pc["eidx16"],
                "nreal": pc["nreal"],
                "dlh": pc["dlh"],
                "iotaC": meta["iotaC"],
                "pohg": pc["pohg"],
                "goff0": pc["goff0"],
                "goff1": pc["goff1"],
                "invc": meta["invc"],
                "fc_W": fc_Wp,
                "fc_b": np.asarray(fc_b, np.float32),
                "ident32": meta["ident32"],
            }
        )
    return in_maps


def prep_and_build(cfg, inputs, debug=False):
    x = np.asarray(inputs["x"], dtype=np.float32)
    edge_index = np.asarray(inputs["edge_index"])
    batch = np.asarray(inputs["batch"]).astype(np.int64)
    meta, per_core = host_prep(cfg, x, edge_index, batch)
    nc = build_program(cfg, meta, debug=debug)
    in_maps = make_in_maps(
        cfg, meta, per_core,
        inputs["W_td"], inputs["b_td"], inputs["W_bu"], inputs["b_bu"],
        inputs["fc_W"], inputs["fc_b"],
    )
    return nc, in_maps


def run(cfg, inputs, debug=False, trace=False):
    nc, in_maps = prep_and_build(cfg, inputs, debug=debug)
    res = run_bass_kernel_spmd(nc, in_maps, list(range(cfg.NC)), trace=trace)
    out = res.results[0]["out"].astype(np.float32)
    return out, res


def full_cfg():
    return Cfg(
        n_nodes=100000, n_graphs=1000, n_cores=8, banks=4,
        in_f=128, hid_f=64, out_f=4, grp=6,
    )


def kernel(**inputs):
    out, _ = run(full_cfg(), inputs)
    return out


# revision 36
# speedup vs baseline: 1.0025x; 1.0025x over previous
"""BiGCN (two fused GCNConv + graph mean-pool + FC + log_softmax) on 8 trn2 cores.

Strategy (graph/data parallel, partitioned by destination node range):
  - core c owns nodes [c*NSH, (c+1)*NSH) as edge destinations
  - host sorts edges into per-(bank, dst-tile) cells padded to 128-slot
    chunks; SWDGE dma_gather per cell with per-core real counts fed via
    gpsimd registers (the ~9ns/idx gpsimd ucode is the hw floor for
    row-granular gathers; HW-DGE queues cannot expand indirect offsets)
  - host precomputes degree-normalization (dinv), pooling one-hots and
    per-graph inverse counts; x is shipped pre-transposed in bf16
  - device: Hn = (xT.T @ [W_td|W_bu]) * dinv (bf16) written per bank
    stripe; bank 0 is small (5 tiles) so its AllGather lands early and
    gathers start ~40us in; per-bank waves gather + scatter-accumulate
    into an SBUF f32 accumulator via one-hot matmuls (one-hot built on
    DVE with packed-last-dim APs); each tile's normalize/feat/pool work
    is fused right after its last bank so the tail hides in the gathers
  - final: out[d] = acc[d]*dinv[d] + (h[d]*dinv[d]^2 + b);
    feat = [relu(td)|relu(bu) | td|bu] (fc_W rows host-permuted),
    graph pooling via host one-hot matmuls, indirect-scatter +
    AllReduce, FC + log_softmax replicated on every core.
"""

import math

import numpy as np
import ml_dtypes

import concourse.bass as bass
import concourse.bacc as bacc
import concourse.mybir as mybir
import concourse.tile as tile
from concourse.bass import IndirectOffsetOnAxis
from concourse.bass_utils import run_bass_kernel_spmd
from concourse.library_config import mlp as mlp_lib

BF16 = mybir.dt.bfloat16
F32 = mybir.dt.float32
I16 = mybir.dt.int16
I32 = mybir.dt.int32
AF = mybir.ActivationFunctionType
ALU = mybir.AluOpType
NPBF = ml_dtypes.bfloat16

P = 128


def _split_even(n, k):
    base = n // k
    rem = n % k
    return [base + (1 if i < rem else 0) for i in range(k)]


class Cfg:
    def __init__(self, n_nodes, n_graphs, n_cores, banks, in_f, hid_f, out_f,
                 grp=14):
        assert n_nodes % n_cores == 0
        self.N = n_nodes
        self.G = n_graphs
        self.NC = n_cores
        self.NSH = n_nodes // n_cores
        self.T = math.ceil(self.NSH / P)
        self.NSH_P = self.T * P
        self.BANKS = min(banks, self.T)
        # small leading stripe so bank-0 gathers start early; later stripes
        # at the int16 table-index cap (n_cores*cap*128 <= 32767)
        cap = 32767 // (n_cores * P)
        if (self.BANKS > 2
                and (self.BANKS - 1) * cap < self.T <= self.BANKS * cap
                and self.T - (self.BANKS - 1) * cap + 2 <= cap):
            lead = self.T - (self.BANKS - 1) * cap + 2
            self.QT = ([lead] + [cap] * (self.BANKS - 2) + [cap - 2])
        elif (self.BANKS > 1
                and (self.BANKS - 1) * cap < self.T <= self.BANKS * cap):
            lead = self.T - (self.BANKS - 1) * cap
            self.QT = [lead] + [cap] * (self.BANKS - 1)
        else:
            self.QT = _split_even(self.T, self.BANKS)  # tiles per stripe
        self.QSTART = np.concatenate([[0], np.cumsum(self.QT)])
        self.QROWS = [q * P for q in self.QT]
        self.IN_F = in_f
        self.HID = hid_f
        self.FW = 2 * hid_f
        assert self.FW == P and in_f == P
        self.OUT_F = out_f
        self.FEAT = 4 * hid_f  # [relu(td)|relu(bu) | td|bu]
        self.GB = math.ceil(self.G / P)
        self.PART_ROWS = (self.G + 2 * P + P - 1) // P * P
        self.GRP = min(grp, self.T)
        self.NGRP = math.ceil(self.T / self.GRP)


def host_prep(cfg, x, edge_index, batch):
    """Sort edges into (bank, dst-tile) cells, build device inputs."""
    c = cfg
    src = edge_index[0].astype(np.int64)
    dst = edge_index[1].astype(np.int64)
    assert src.min() >= 0 and src.max() < c.N
    assert dst.min() >= 0 and dst.max() < c.N

    # degree incl. self-loop, symmetric normalization
    deg = np.bincount(dst, minlength=c.N).astype(np.float64) + 1.0
    dinv = 1.0 / np.sqrt(deg)  # [N]

    # source row in its bank table (bank k rows: core-major stripes)
    qstart_rows = c.QSTART[:-1] * P
    sc, so = np.divmod(src, c.NSH)
    stile = so // P
    sbank = np.searchsorted(c.QSTART[1:], stile, side="right")
    qrows_arr = np.asarray(c.QROWS)
    lidx = (sc * qrows_arr[sbank] + (so - qstart_rows[sbank])).astype(np.int64)

    owner, do = np.divmod(dst, c.NSH)
    tloc = do // P
    dl = do % P

    # cells ordered (owner, bank, tile); packed at 16-slot granules
    ncell = c.NC * c.BANKS * c.T
    cell = (owner * c.BANKS + sbank) * c.T + tloc
    order = np.argsort(cell, kind="stable")
    cell_s = cell[order]
    lidx_s = lidx[order]
    dl_s = dl[order]
    tloc_s = tloc[order]
    counts = np.bincount(cell_s, minlength=ncell).reshape(c.NC, c.BANKS, c.T)
    # cell capacity in slots: 16-granular, >=128 so a 128-chunk never spans
    # more than two (adjacent-tile) cells
    S = np.maximum((-(-counts // 16)).max(axis=0) * 16, P)  # [BANKS, T]

    starts = np.zeros(ncell + 1, dtype=np.int64)
    np.cumsum(counts.reshape(-1), out=starts[1:])
    rank = np.arange(len(cell_s), dtype=np.int64) - starts[cell_s]

    # calls: (bank, tile-group); cells packed back-to-back inside a call
    calls = []  # dicts: j, t0, t1, L, CH, slot_base, ebase, dbase, win, o
    slot_base = 0
    ecols = 0
    dcols = 0
    for j in range(c.BANKS):
        t = 0
        while t < c.T:
            t1 = min(t + c.GRP, c.T)
            o = np.zeros(t1 - t + 1, dtype=np.int64)
            np.cumsum(S[j, t:t1], out=o[1:])
            L = int(o[-1])
            CH = -(-L // P)
            # window base tile per chunk: the cell containing slot q*128
            win = np.searchsorted(o[1:], np.arange(CH) * P, side="right") + t
            calls.append(dict(j=j, t0=t, t1=t1, L=L, CH=CH,
                              slot_base=slot_base, ebase=ecols, dbase=dcols,
                              win=win, o=o))
            slot_base += CH * P  # call slot spaces are chunk-padded
            ecols += L // 16
            dcols += CH
            t = t1
    SLOT_TOT = slot_base
    ECOLS = max(ecols, 8)
    DCOLS = max(dcols, 1)
    CHMAX = max((cl["CH"] for cl in calls), default=1)

    # per-edge global slot position and window-relative dst id
    cell_jt = cell_s % (c.BANKS * c.T)
    jj = cell_jt // c.T
    tt_ = cell_jt % c.T
    call_of_tile = np.zeros((c.BANKS, c.T), dtype=np.int64)
    for ci, cl in enumerate(calls):
        call_of_tile[cl["j"], cl["t0"]:cl["t1"]] = ci
    call_id = call_of_tile[jj, tt_]
    cbase = np.array([cl["slot_base"] for cl in calls], dtype=np.int64)
    ct0 = np.array([cl["t0"] for cl in calls], dtype=np.int64)
    ocum = np.zeros((c.BANKS, c.T), dtype=np.int64)  # cell offset in call
    for cl in calls:
        ocum[cl["j"], cl["t0"]:cl["t1"]] = cl["o"][:-1]
    slot_in_call = ocum[jj, tt_] + rank
    slotpos = cbase[call_id] + slot_in_call
    winflat = np.concatenate([cl["win"] for cl in calls]) if calls else \
        np.zeros(1, np.int64)
    chbase = np.zeros(len(calls), dtype=np.int64)
    acc_ch = 0
    for ci, cl in enumerate(calls):
        chbase[ci] = acc_ch
        acc_ch += cl["CH"]
    ta_of_edge = winflat[chbase[call_id] + slot_in_call // P]
    dl2_s = dl_s + P * (tloc_s - ta_of_edge)
    assert dl2_s.min() >= 0 and dl2_s.max() < 2 * P, "chunk window overflow"

    g_base = np.empty(c.NC, dtype=np.int64)
    cnt_g = np.bincount(batch, minlength=c.G).astype(np.float64)

    per_core = []
    for cc in range(c.NC):
        e0, e1 = starts[cc * c.BANKS * c.T], starts[(cc + 1) * c.BANKS * c.T]
        eflat = np.full(SLOT_TOT, -1, dtype=np.int64)
        dflat = np.full(SLOT_TOT, 500.0, dtype=np.float32)
        eflat[slotpos[e0:e1]] = lidx_s[e0:e1]
        dflat[slotpos[e0:e1]] = dl2_s[e0:e1]
        # dl2 per (partition, chunk-col): [P, DCOLS]
        dlh = np.ascontiguousarray(
            dflat.reshape(-1, P).T
        ).astype(NPBF)

        eidx16 = np.full((P, ECOLS), -1, dtype=np.int16)
        nreal = np.zeros((1, -(-max(len(calls), 1) // 4) * 4),
                         dtype=np.int32)
        for ci, cl in enumerate(calls):
            sb, L = cl["slot_base"], cl["L"]
            li = eflat[sb : sb + L].copy()
            nz = np.flatnonzero(li != -1)
            if len(nz) == 0:
                li[0] = 0
                n = 1
            else:
                # mid pads must be valid indices (ucode forbids interior
                # -1); point them at row 0, zero-weighted by the one-hot
                last = int(nz[-1])
                seg = li[: last + 1]
                seg[seg == -1] = 0
                n = last + 1
            w = li.reshape(L // 16, 16).T.astype(np.int16)
            eb = cl["ebase"]
            eidx16[:, eb : eb + L // 16] = np.tile(w, (8, 1))
            nreal[0, ci] = n

        xs = np.zeros((c.NSH_P, c.IN_F), dtype=np.float32)
        xs[: c.NSH] = x[cc * c.NSH : (cc + 1) * c.NSH]
        xT = np.ascontiguousarray(xs.T).astype(NPBF)  # [IN_F, NSH_P]

        dv = np.zeros(c.NSH_P, dtype=np.float32)
        dv[: c.NSH] = dinv[cc * c.NSH : (cc + 1) * c.NSH]
        dinvT = np.ascontiguousarray(dv.reshape(c.T, P).T)
        dinv2T = dinvT * dinvT

        b = batch[cc * c.NSH : (cc + 1) * c.NSH]
        g_base[cc] = int(b[0])
        assert int(b[-1]) - int(b[0]) < 2 * P, "graph span exceeds 2 blocks"
        brel = np.full(c.NSH_P, 300, dtype=np.int64)
        brel[: c.NSH] = b - g_base[cc]
        pohg = np.zeros((c.NSH_P, 2 * P), dtype=np.float32)
        valid = brel < 2 * P
        pohg[np.arange(c.NSH_P)[valid], brel[valid]] = 1.0
        # [P, T*2P]: tile-major blocks, slot p on partition p
        pohgT = np.ascontiguousarray(
            pohg.reshape(c.T, P, 2 * P).transpose(1, 0, 2).reshape(P, -1)
        ).astype(NPBF)

        goff0 = (g_base[cc] + np.arange(P)).astype(np.int32).reshape(P, 1)
        goff1 = goff0 + P
        per_core.append(
            dict(xT=xT, eidx16=eidx16, nreal=nreal, dlh=dlh, pohg=pohgT,
                 dinvT=dinvT, dinv2T=dinv2T, goff0=goff0, goff1=goff1)
        )

    # shared constants
    iotaC = np.repeat(np.arange(2 * P, dtype=np.float32), CHMAX)
    iotaC = np.tile(iotaC, (P, 1)).astype(NPBF)  # [P, 2P*CHMAX]
    invc = np.ones((P, c.GB), dtype=np.float32)
    for bb in range(c.GB):
        gs = np.arange(bb * P, min((bb + 1) * P, c.G))
        invc[: len(gs), bb] = 1.0 / np.maximum(cnt_g[gs], 1.0)
    ident32 = np.eye(P, dtype=np.float32)

    meta = dict(calls=calls, S=S, ECOLS=ECOLS, DCOLS=DCOLS, CHMAX=CHMAX,
                iotaC=iotaC, invc=invc, ident32=ident32)
    return meta, per_core


def build_program(cfg, meta, debug=False):
    c = cfg
    calls = meta["calls"]
    ECOLS = meta["ECOLS"]
    DCOLS = meta["DCOLS"]
    CHMAX = meta["CHMAX"]
    H = c.HID
    FW = c.FW
    FEAT = c.FEAT
    NCALL = -(-max(len(calls), 1) // 4) * 4

    nc = bacc.Bacc(
        "TRN2", target_bir_lowering=False, debug=debug, num_devices=c.NC
    )

    # ---- I/O ----
    xT_d = nc.dram_tensor("xT", [P, c.NSH_P], BF16, kind="ExternalInput")
    wcat_d = nc.dram_tensor("wcat", [c.IN_F, FW], BF16, kind="ExternalInput")
    bias_d = nc.dram_tensor("bias_bc", [P, FW], BF16, kind="ExternalInput")
    dinvT_d = nc.dram_tensor("dinvT", [P, c.T], F32, kind="ExternalInput")
    dinv2T_d = nc.dram_tensor("dinv2T", [P, c.T], F32, kind="ExternalInput")
    eidx_d = nc.dram_tensor("eidx16", [P, ECOLS], I16, kind="ExternalInput")
    nreal_d = nc.dram_tensor("nreal", [1, NCALL], I32, kind="ExternalInput")
    dl_d = nc.dram_tensor("dlh", [P, DCOLS], BF16, kind="ExternalInput")
    iota_d = nc.dram_tensor("iotaC", [P, 2 * P * CHMAX], BF16,
                            kind="ExternalInput")
    pohg_d = nc.dram_tensor("pohg", [P, c.T * 2 * P], BF16, kind="ExternalInput")
    goff0_d = nc.dram_tensor("goff0", [P, 1], I32, kind="ExternalInput")
    goff1_d = nc.dram_tensor("goff1", [P, 1], I32, kind="ExternalInput")
    invc_d = nc.dram_tensor("invc", [P, c.GB], F32, kind="ExternalInput")
    fcW_d = nc.dram_tensor("fc_W", [2 * FW, c.OUT_F], F32, kind="ExternalInput")
    fcb_d = nc.dram_tensor("fc_b", [c.OUT_F], F32, kind="ExternalInput")
    ident_d = nc.dram_tensor("ident32", [P, P], F32, kind="ExternalInput")
    out = nc.dram_tensor("out", [c.G, c.OUT_F], F32, kind="ExternalOutput")

    # ---- internal DRAM ----
    hn_local = nc.dram_tensor("hn_local", [c.NSH_P, FW], BF16)
    hn_q = [
        nc.dram_tensor(f"hn_q{k}", [c.NC * c.QROWS[k], FW], BF16,
                       addr_space="Shared")
        for k in range(c.BANKS)
    ]
    partial = nc.dram_tensor("partial", [c.PART_ROWS, FEAT], BF16)
    total = nc.dram_tensor("total", [c.PART_ROWS, FEAT], BF16,
                           addr_space="Shared")

    groups = [list(range(c.NC))]

    with tile.TileContext(nc) as tc:
        with (
            tc.tile_pool(name="const", bufs=1) as cp,
            tc.tile_pool(name="sb", bufs=3) as sp,
            tc.tile_pool(name="oh", bufs=2) as op_,
            tc.tile_pool(name="strip", bufs=4) as gp,
            nc.gpsimd.register("nr0") as r0,
            nc.gpsimd.register("nr1") as r1,
            nc.gpsimd.register("nr2") as r2,
            nc.gpsimd.register("nr3") as r3,
        ):
            regs = [r0, r1, r2, r3]
            nc.gpsimd.load_library(mlp_lib)

            # ---- constants ----
            wcat = cp.tile([P, FW], BF16)
            nc.sync.dma_start(wcat[:], wcat_d[:])
            bias_sb = cp.tile([P, FW], BF16)
            nc.sync.dma_start(bias_sb[:], bias_d[:])
            dinvT = cp.tile([P, c.T], F32)
            nc.sync.dma_start(dinvT[:], dinvT_d[:])
            dinv2T = cp.tile([P, c.T], F32)
            nc.sync.dma_start(dinv2T[:], dinv2T_d[:])
            eidx_sb = cp.tile([P, ECOLS], I16)
            nc.scalar.dma_start(eidx_sb[:], eidx_d[:])
            nreal_sb = cp.tile([1, NCALL], I32)
            nc.scalar.dma_start(nreal_sb[:], nreal_d[:])
            dl_sb = cp.tile([P, DCOLS], BF16)
            nc.scalar.dma_start(dl_sb[:], dl_d[:])
            iota_sb = cp.tile([P, 2 * P * CHMAX], BF16)
            nc.scalar.dma_start(iota_sb[:], iota_d[:])
            goff0_sb = cp.tile([P, 1], I32)
            nc.sync.dma_start(goff0_sb[:], goff0_d[:])
            goff1_sb = cp.tile([P, 1], I32)
            nc.sync.dma_start(goff1_sb[:], goff1_d[:])
            invc_sb = cp.tile([P, c.GB], F32)
            nc.sync.dma_start(invc_sb[:], invc_d[:])
            fw0 = cp.tile([P, c.OUT_F], F32)
            nc.sync.dma_start(fw0[:], fcW_d[0:P, :])
            fw1 = cp.tile([P, c.OUT_F], F32)
            nc.sync.dma_start(fw1[:], fcW_d[P : 2 * P, :])
            fcb = cp.tile([c.OUT_F, 1], F32)
            nc.sync.dma_start(fcb[:, 0:1], fcb_d[:, None])
            ident32 = cp.tile([P, P], F32)
            nc.sync.dma_start(ident32[:], ident_d[:])

            hb_all = cp.tile([P, c.T * FW], BF16)   # h*dinv^2 + bias
            acc_all = cp.tile([P, c.T * FW], BF16)  # edge-message sums

            # ---- phase 1: Hn per bank stripe, AllGather each stripe ----
            with tc.tile_pool(name="ps1", bufs=2, space="PSUM") as pp:
                for k in range(c.BANKS):
                    for t in range(int(c.QSTART[k]), int(c.QSTART[k + 1])):
                        xt = sp.tile([P, P], BF16, tag="xt")
                        nc.sync.dma_start(xt[:], xT_d[:, t * P : (t + 1) * P])
                        h_ps = pp.tile([P, FW], F32, space="PSUM", tag="h")
                        nc.tensor.matmul(
                            h_ps[:], lhsT=xt[:], rhs=wcat[:],
                            start=True, stop=True,
                        )
                        hn_sb = sp.tile([P, FW], BF16, tag="hn")
                        nc.scalar.activation(
                            hn_sb[:], h_ps[:], AF.Copy,
                            scale=dinvT[:, t : t + 1],
                        )
                        nc.sync.dma_start(
                            hn_local[t * P : (t + 1) * P, :], hn_sb[:]
                        )
                        hb0 = sp.tile([P, FW], BF16, tag="hb0")
                        nc.scalar.activation(
                            hb0[:], h_ps[:], AF.Copy,
                            scale=dinv2T[:, t : t + 1],
                        )
                        nc.vector.tensor_tensor(
                            out=hb_all[:, t * FW : (t + 1) * FW],
                            in0=hb0[:], in1=bias_sb[:], op=ALU.add,
                        )
                    r_lo = int(c.QSTART[k]) * P
                    nc.gpsimd.collective_compute(
                        "AllGather",
                        ALU.bypass,
                        ins=[hn_local[r_lo : r_lo + c.QROWS[k], :]],
                        outs=[hn_q[k][:]],
                        replica_groups=groups,
                    )

            # zero the pooling partial buffer early (SP queue, off critical path)
            zt = sp.tile([P, FEAT], BF16, tag="zt")
            nc.vector.memset(zt[:], 0.0)
            for r in range(0, c.PART_ROWS, P):
                nc.sync.dma_start(partial[r : r + P, :], zt[:])

            # ---- scatter waves: per bank, merged tile-group gather calls;
            # ---- per-tile normalize/feat/pool fused after its last bank ----
            first = [True] * c.T
            fin_first = 0
            fin_last = c.T - 1

            def finalize_tile(t):
                poh = sp.tile([P, 2 * P], BF16, tag="poh")
                nc.sync.dma_start(
                    poh[:], pohg_d[:, t * 2 * P : (t + 1) * 2 * P]
                )
                s1t = sp.tile([P, FW], F32, tag="s1")
                nc.scalar.activation(
                    s1t[:], acc_all[:, t * FW : (t + 1) * FW],
                    AF.Copy, scale=dinvT[:, t : t + 1],
                )
                ot = sp.tile([P, FW], F32, tag="ot")
                nc.vector.tensor_tensor(
                    out=ot[:], in0=s1t[:],
                    in1=hb_all[:, t * FW : (t + 1) * FW], op=ALU.add,
                )
                feat = sp.tile([P, FEAT], BF16, tag="feat")
                nc.scalar.activation(feat[:, 0:FW], ot[:], AF.Relu)
                nc.scalar.activation(feat[:, FW:FEAT], ot[:], AF.Copy)
                nc.tensor.matmul(
                    pool0[:], lhsT=poh[:, 0:P], rhs=feat[:],
                    start=(t == fin_first), stop=(t == fin_last),
                )
                nc.tensor.matmul(
                    pool1[:], lhsT=poh[:, P : 2 * P], rhs=feat[:],
                    start=(t == fin_first), stop=(t == fin_last),
                )

            n_gather = 0
            GT_BUFS = 4
            with (
                tc.tile_pool(name="psP", bufs=1, space="PSUM") as pa,
                tc.tile_pool(name="ps4", bufs=3, space="PSUM") as pp,
            ):
                pool0 = pa.tile([P, FEAT], F32, space="PSUM")
                pool1 = pa.tile([P, FEAT], F32, space="PSUM")
                for ci, cl in enumerate(calls):
                    j, t0, t1 = cl["j"], cl["t0"], cl["t1"]
                    L, CH = cl["L"], cl["CH"]
                    o, win = cl["o"], cl["win"]
                    gt_t = gp.tile([P, CHMAX * P], BF16, tag="gt")
                    # pad slots are never written by the gather: scrub each
                    # pool slot once at uniform max extent; afterwards every
                    # byte is scrub-zero or old gather data (finite), so no
                    # NaN reaches the zero-weighted matmuls
                    if n_gather < GT_BUFS:
                        nc.vector.memset(gt_t[:], 0.0)
                    if n_gather % 4 == 0:
                        nc.gpsimd.reg_load(
                            regs, nreal_sb[0:1, ci : ci + 4]
                        )
                    reg = regs[n_gather % 4]
                    eb = cl["ebase"]
                    nc.gpsimd.dma_gather(
                        gt_t[:, 0 : CH * P].rearrange(
                            "p (b e) -> p b e", e=FW
                        ),
                        hn_q[j][:],
                        eidx_sb[:, eb : eb + L // 16],
                        L,
                        reg,
                        FW,
                        single_packet=(L <= 1024),
                    )
                    n_gather += 1
                    db = cl["dbase"]
                    oh = op_.tile([P, 2 * P * CH], BF16, tag="oh")
                    nc.vector.tensor_tensor(
                        out=oh[:].rearrange("p (d g) -> p d g", g=CH),
                        in0=dl_sb[:, db : db + CH]
                        .unsqueeze(1)
                        .broadcast_to([P, 2 * P, CH]),
                        in1=iota_sb[:]
                        .rearrange("p (d g) -> p d g", g=CHMAX)[:, :, 0:CH],
                        op=ALU.is_equal,
                    )
                    oh3 = oh[:].rearrange("p (d g) -> p d g", g=CH)
                    for t in range(t0, t1):
                        q_lo = int(o[t - t0]) // P
                        q_hi = (int(o[t - t0 + 1]) - 1) // P
                        acc = pp.tile([P, FW], F32, space="PSUM", tag="acc")
                        for q in range(q_lo, q_hi + 1):
                            assert win[q] in (t, t - 1), (win[q], t)
                            half = 0 if win[q] == t else 1
                            nc.tensor.matmul(
                                acc[:],
                                lhsT=oh3[:, half * P : (half + 1) * P,
                                         q : q + 1],
                                rhs=gt_t[:, q * P : (q + 1) * P],
                                start=(q == q_lo),
                                stop=(q == q_hi),
                            )
                        dst_sl = acc_all[:, t * FW : (t + 1) * FW]
                        if first[t]:
                            nc.vector.tensor_copy(dst_sl, acc[:])
                            first[t] = False
                        else:
                            nc.vector.tensor_tensor(
                                out=dst_sl, in0=acc[:], in1=dst_sl,
                                op=ALU.add,
                            )
                        if j == c.BANKS - 1:
                            finalize_tile(t)

                pp0 = sp.tile([P, FEAT], BF16, tag="pp0")
                nc.vector.tensor_copy(pp0[:], pool0[:])
                nc.gpsimd.indirect_dma_start(
                    out=partial[:],
                    out_offset=IndirectOffsetOnAxis(ap=goff0_sb[:, 0:1],
                                                    axis=0),
                    in_=pp0[:],
                    in_offset=None,
                )
                pp1 = sp.tile([P, FEAT], BF16, tag="pp1")
                nc.vector.tensor_copy(pp1[:], pool1[:])
                nc.gpsimd.indirect_dma_start(
                    out=partial[:],
                    out_offset=IndirectOffsetOnAxis(ap=goff1_sb[:, 0:1],
                                                    axis=0),
                    in_=pp1[:],
                    in_offset=None,
                )

            # ---- AllReduce pooled sums ----
            nc.gpsimd.collective_compute(
                "AllReduce",
                ALU.add,
                ins=[partial[:]],
                outs=[total[:]],
                replica_groups=groups,
            )

            # ---- mean, FC, log_softmax (replicated) ----
            # two passes so the scalar engine loads the Exp/Ln activation
            # tables once instead of per graph-block
            OUTF = c.OUT_F
            ls_all = cp.tile([P, c.GB * OUTF], F32)
            ex_all = cp.tile([P, c.GB * OUTF], F32)
            sm_all = cp.tile([P, c.GB], F32)
            with tc.tile_pool(name="ps7", bufs=2, space="PSUM") as pp:
                for b in range(c.GB):
                    tt = sp.tile([P, FEAT], BF16, tag="tt")
                    nc.sync.dma_start(tt[:], total[b * P : (b + 1) * P, :])
                    mean_sb = sp.tile([P, FEAT], F32, tag="mean")
                    nc.vector.tensor_scalar(
                        out=mean_sb[:], in0=tt[:],
                        scalar1=invc_sb[:, b : b + 1], scalar2=None,
                        op0=ALU.mult,
                    )
                    lg_ps = pp.tile([P, P], F32, space="PSUM", tag="lg")
                    for half in range(2):
                        tp_ps = pp.tile([P, P], F32, space="PSUM", tag="tp")
                        nc.tensor.transpose(
                            tp_ps[:], mean_sb[:, half * P : (half + 1) * P],
                            ident32[:],
                        )
                        mt = sp.tile([P, P], F32, tag="mt")
                        nc.vector.tensor_copy(mt[:], tp_ps[:])
                        nc.tensor.matmul(
                            lg_ps[0:OUTF, :],
                            lhsT=(fw0 if half == 0 else fw1)[:],
                            rhs=mt[:],
                            start=(half == 0),
                            stop=(half == 1),
                        )
                    lgb = sp.tile([OUTF, P], F32, tag="lgb")
                    nc.vector.tensor_scalar(
                        out=lgb[:], in0=lg_ps[0:OUTF, :],
                        scalar1=fcb[:, 0:1], scalar2=None, op0=ALU.add,
                    )
                    tr_ps = pp.tile([P, OUTF], F32, space="PSUM", tag="tr")
                    nc.tensor.transpose(
                        tr_ps[:], lgb[:], ident32[0:OUTF, 0:OUTF]
                    )
                    sl = ls_all[:, b * OUTF : (b + 1) * OUTF]
                    nc.vector.tensor_copy(sl, tr_ps[:])
                    mx = sp.tile([P, 1], F32, tag="mx")
                    nc.vector.reduce_max(mx[:], sl, axis=mybir.AxisListType.X)
                    nc.vector.tensor_scalar(
                        out=sl, in0=sl, scalar1=mx[:, 0:1], scalar2=None,
                        op0=ALU.subtract,
                    )
                for b in range(c.GB):
                    nc.scalar.activation(
                        ex_all[:, b * OUTF : (b + 1) * OUTF],
                        ls_all[:, b * OUTF : (b + 1) * OUTF], AF.Exp,
                    )
                for b in range(c.GB):
                    nc.vector.reduce_sum(
                        sm_all[:, b : b + 1],
                        ex_all[:, b * OUTF : (b + 1) * OUTF],
                        axis=mybir.AxisListType.X,
                    )
                for b in range(c.GB):
                    nc.scalar.activation(
                        sm_all[:, b : b + 1], sm_all[:, b : b + 1], AF.Ln
                    )
                for b in range(c.GB):
                    h_rows = min(P, c.G - b * P)
                    ls = sp.tile([P, OUTF], F32, tag="ls")
                    nc.vector.tensor_scalar(
                        out=ls[:], in0=ls_all[:, b * OUTF : (b + 1) * OUTF],
                        scalar1=sm_all[:, b : b + 1], scalar2=None,
                        op0=ALU.subtract,
                    )
                    nc.sync.dma_start(
                        out[b * P : b * P + h_rows, :], ls[0:h_rows, :]
                    )

    nc.compile()
    return nc


def make_in_maps(cfg, meta, per_core, W_td, b_td, W_bu, b_bu, fc_W, fc_b):
    c = cfg
    H = c.HID
    wcat = np.concatenate(
        [np.asarray(W_td, np.float32), np.asarray(W_bu, np.float32)], axis=1
    ).astype(NPBF)
    bias = np.concatenate(
        [np.asarray(b_td, np.float32), np.asarray(b_bu, np.float32)]
    )
    bias_bc = np.tile(bias.reshape(1, -1), (P, 1)).astype(NPBF)
    # feat order [relu_td, relu_bu, td, bu] -> permute fc_W rows
    fw = np.asarray(fc_W, np.float32)
    perm = np.concatenate([
        np.arange(0, H),            # relu(td)
        np.arange(2 * H, 3 * H),    # relu(bu)
        np.arange(H, 2 * H),        # td
        np.arange(3 * H, 4 * H),    # bu
    ])
    fc_Wp = np.ascontiguousarray(fw[perm])

    in_maps = []
    for cc in range(c.NC):
        pc = per_core[cc]
        in_maps.append(
            {
                "xT": pc["xT"],
                "wcat": wcat,
                "bias_bc": bias_bc,
                "dinvT": pc["dinvT"],
                "dinv2T": pc["dinv2T"],
                "eidx16": pc["eidx16"],
                "nreal": pc["nreal"],
                "dlh": pc["dlh"],
                "iotaC": meta["iotaC"],
                "pohg": pc["pohg"],
                "goff0": pc["goff0"],
                "goff1": pc["goff1"],
                "invc": meta["invc"],
                "fc_W": fc_Wp,
                "fc_b": np.asarray(fc_b, np.float32),
                "ident32": meta["ident32"],
            }
        )
    return in_maps


def prep_and_build(cfg, inputs, debug=False):
    x = np.asarray(inputs["x"], dtype=np.float32)
    edge_index = np.asarray(inputs["edge_index"])
    batch = np.asarray(inputs["batch"]).astype(np.int64)
    meta, per_core = host_prep(cfg, x, edge_index, batch)
    nc = build_program(cfg, meta, debug=debug)
    in_maps = make_in_maps(
        cfg, meta, per_core,
        inputs["W_td"], inputs["b_td"], inputs["W_bu"], inputs["b_bu"],
        inputs["fc_W"], inputs["fc_b"],
    )
    return nc, in_maps


def run(cfg, inputs, debug=False, trace=False):
    nc, in_maps = prep_and_build(cfg, inputs, debug=debug)
    res = run_bass_kernel_spmd(nc, in_maps, list(range(cfg.NC)), trace=trace)
    out = res.results[0]["out"].astype(np.float32)
    return out, res


def full_cfg():
    return Cfg(
        n_nodes=100000, n_graphs=1000, n_cores=8, banks=4,
        in_f=128, hid_f=64, out_f=4, grp=7,
    )


def kernel(**inputs):
    out, _ = run(full_cfg(), inputs)
    return out
